# revision 2
# baseline (speedup 1.0000x reference)
"""Trainium2 Bass kernel for nn_ClusteringLayer (vq_codebook, Student-t assignments).

Computes, for x in R^{N x D} and clusters c in R^{K x D}:
    d2[n,k] = ||x_n - c_k||^2
    q = 1 / (1 + d2)            (Student-t, alpha=1, power=(alpha+1)/2=1)
    out = q / q.sum(-1, keepdims=True)

Strategy (data-parallel over 8 NeuronCores, cluster table replicated):
  - host: shard x along N (65536 rows/core), transpose+cast each shard to
    bf16 X^T [D, Nsh] so the contraction dim lands on SBUF partitions.
  - device per 1024-column block:
      psum[k, n] = (-2C)^T.T @ X^T  +  ones.T @ (X^T)^2        (4 matmuls)
      u = Ln(psum + (1 + ||c||^2))   [ACT, per-partition bias, fp16 out]
      u^T via 8x DMA-xbar transposes  -> [n, k] layout
      q = Exp(-u^T)  [ACT, bf16]
      s = row-sum (DVE reduce), out = q * (1/s)  (DVE), store bf16
  - host: upcast bf16 -> f32, concat shards.
"""

import numpy as np
from contextlib import ExitStack

N, D, K = 524288, 256, 64
NCORES = 8
NSH = N // NCORES  # 65536 rows per core
BLK = 1024         # n-columns per block
TP = BLK // 128    # transpose slabs per block (8)


def _build(nsh=NSH, blk=BLK):
    import concourse.bacc as bacc
    import concourse.tile as tile
    from concourse import mybir

    f32 = mybir.dt.float32
    bf16 = mybir.dt.bfloat16
    f16 = mybir.dt.float16
    nblk = nsh // blk
    tp = blk // 128

    nc = bacc.Bacc("TRN2", target_bir_lowering=False, debug=False)
    xt = nc.dram_tensor("xt", [D, nsh], bf16, kind="ExternalInput").ap()
    cl = nc.dram_tensor("clusters", [K, D], f32, kind="ExternalInput").ap()
    qo = nc.dram_tensor("q", [nsh, K], bf16, kind="ExternalOutput").ap()

    with tile.TileContext(nc) as tc, ExitStack() as ctx:
        wp = ctx.enter_context(tc.tile_pool(name="w", bufs=1))
        io = ctx.enter_context(tc.tile_pool(name="io", bufs=3))
        sqp = ctx.enter_context(tc.tile_pool(name="sq", bufs=3))
        up = ctx.enter_context(tc.tile_pool(name="u", bufs=3))
        qp = ctx.enter_context(tc.tile_pool(name="qp", bufs=3))
        sp = ctx.enter_context(tc.tile_pool(name="sp", bufs=4))
        pp = ctx.enter_context(tc.tile_pool(name="ps", bufs=2, space="PSUM"))

        # --- one-time cluster prep (replicated on every core) ---
        cl_sb = wp.tile([K, D], f32, tag="cl")
        nc.sync.dma_start(cl_sb, cl)
        csq = wp.tile([K, D], f32, tag="csq")
        nc.vector.tensor_mul(csq, cl_sb, cl_sb)
        c2 = wp.tile([K, 1], f32, tag="c2")
        nc.vector.tensor_reduce(c2, csq, axis=mybir.AxisListType.X,
                                op=mybir.AluOpType.add)
        c2p1 = wp.tile([K, 1], f32, tag="c2p1")
        nc.vector.tensor_scalar_add(c2p1, c2, 1.0)
        cn2 = wp.tile([K, D], bf16, tag="cn2")
        nc.vector.tensor_scalar_mul(cn2, cl_sb, -2.0)
        ct0 = wp.tile([128, K], bf16, tag="ct0")
        ct1 = wp.tile([128, K], bf16, tag="ct1")
        nc.sync.dma_start_transpose(ct0, cn2[:, 0:128])
        nc.sync.dma_start_transpose(ct1, cn2[:, 128:256])
        ones = wp.tile([128, K], bf16, tag="ones")
        nc.vector.memset(ones, 1.0)

        for b in range(nblk):
            n0 = b * blk
            xt0 = io.tile([128, blk], bf16, tag="xt0")
            xt1 = io.tile([128, blk], bf16, tag="xt1")
            nc.sync.dma_start(xt0, xt[0:128, n0:n0 + blk])
            nc.sync.dma_start(xt1, xt[128:256, n0:n0 + blk])
            xq0 = sqp.tile([128, blk], bf16, tag="xq0")
            xq1 = sqp.tile([128, blk], bf16, tag="xq1")
            nc.gpsimd.tensor_mul(xq0, xt0, xt0)
            nc.vector.tensor_mul(xq1, xt1, xt1)

            ps = pp.tile([K, blk], f32, tag="d2")
            for h in range(blk // 512):
                sl = slice(h * 512, h * 512 + 512)
                nc.tensor.matmul(ps[:, sl], ct0, xt0[:, sl], start=True, stop=False)
                nc.tensor.matmul(ps[:, sl], ct1, xt1[:, sl], start=False, stop=False)
                nc.tensor.matmul(ps[:, sl], ones, xq0[:, sl], start=False, stop=False)
                nc.tensor.matmul(ps[:, sl], ones, xq1[:, sl], start=False, stop=True)

            # u = ln(1 + d2) ; bias adds (1 + ||c_k||^2) per partition k
            u = up.tile([K, blk], f16, tag="u")
            nc.scalar.activation(u, ps, func=mybir.ActivationFunctionType.Ln,
                                 bias=c2p1, scale=1.0)

            # transpose to [n, k] layout: ut[p, t, k] = u[k, 128*t + p]
            ut = up.tile([128, tp, K], f16, tag="ut")
            for t in range(tp):
                nc.sync.dma_start_transpose(ut[:, t, :], u[:, t * 128:(t + 1) * 128])

            # q = exp(-u)
            qb = qp.tile([128, tp, K], bf16, tag="qb")
            nc.scalar.activation(qb, ut,
                                 func=mybir.ActivationFunctionType.Exp, scale=-1.0)

            s = sp.tile([128, tp], f32, tag="s")
            nc.vector.tensor_reduce(s, qb, axis=mybir.AxisListType.X,
                                    op=mybir.AluOpType.add)
            sinv = sp.tile([128, tp], f32, tag="sinv")
            nc.vector.reciprocal(sinv, s)

            qn = qp.tile([128, tp, K], bf16, tag="qn")
            for t in range(tp):
                nc.vector.tensor_scalar_mul(qn[:, t, :], qb[:, t, :],
                                            sinv[:, t:t + 1])

            q_blk = qo[n0:n0 + blk, :].rearrange("(t p) k -> p t k", p=128)
            nc.sync.dma_start(q_blk, qn)

    nc.compile()
    return nc


_CACHE = {}


def _get_nc(nsh=NSH, blk=BLK):
    key = (nsh, blk)
    if key not in _CACHE:
        _CACHE[key] = _build(nsh, blk)
    return _CACHE[key]


def kernel(inputs, clusters):
    import ml_dtypes
    from concourse.bass_utils import run_bass_kernel_spmd

    x = np.asarray(inputs)
    c = np.ascontiguousarray(np.asarray(clusters, dtype=np.float32))
    assert x.shape == (N, D) and c.shape == (K, D)

    nc = _get_nc()
    xb = x.astype(ml_dtypes.bfloat16)
    in_maps = []
    for i in range(NCORES):
        xts = np.ascontiguousarray(xb[i * NSH:(i + 1) * NSH].T)  # [D, NSH] bf16
        in_maps.append({"xt": xts, "clusters": c})

    res = run_bass_kernel_spmd(nc, in_maps, core_ids=list(range(NCORES)))
    out = np.concatenate(
        [np.asarray(r["q"]).astype(np.float32) for r in res.results], axis=0
    )
    return out


# revision 3
# speedup vs baseline: 2.1236x; 2.1236x over previous
"""Trainium2 Bass kernel for nn_ClusteringLayer (vq_codebook, Student-t assignments).

Computes, for x in R^{N x D} and clusters c in R^{K x D}:
    d2[n,k] = ||x_n - c_k||^2
    q = 1 / (1 + d2)            (Student-t, alpha=1, power=(alpha+1)/2=1)
    out = q / q.sum(-1, keepdims=True)

Strategy (data-parallel over 8 NeuronCores, cluster table replicated):
  - host: shard x along N (65536 rows/core), transpose+cast each shard to
    bf16 X^T [D, Nsh] so the contraction dim lands on SBUF partitions.
  - device, per 1024-column block, with C^T stationary on the PE:
      psum[k, n] = (-2C)^T.T @ X^T  +  ones.T @ (X^T)^2     (8 matmuls of 512)
      u[k, n]    = Ln(psum + (1 + ||c||^2))   [ACT; per-partition bias; fp16]
      u^T        = one DMA-xbar transpose  -> [n, k] layout [128, 8, 64]
      q          = Exp(-u^T)                 [ACT; bf16]
      s          = row-sum over k (DVE reduce), out = q * (1/s) (DVE), store.
  - host: upcast bf16 -> f32, concat shards.
"""

import numpy as np
from contextlib import ExitStack

N, D, K = 524288, 256, 64
NCORES = 8
NSH = N // NCORES  # 65536 rows per core
BLK = 1024         # n-columns per block
# DMA-xbar transpose of [64, BLK] -> [128, BLK//128, 64]: which logical row
# ordering the xbar produces.  "tp" => row r = t*128 + p ; "pt" => r = p*tp + t
XBAR_ORDER = "tp"


def _patch_act_tables():
    """Make Ln and Exp resolve to the single set that contains both
    (natural_log_exp_and_others), so the kernel pays one ACT_TABLE_LOAD
    instead of alternating sets every block.  Only values are modified --
    set order (and hence act_func_set_id indices) is preserved."""
    import functools
    from concourse import hw_specs, bacc, mybir

    if getattr(hw_specs, "_act_tables_patched", False):
        return
    orig = hw_specs.get_activation_tables

    @functools.cache
    def patched(arch):
        t = dict(orig(arch))
        ln = mybir.ActivationFunctionType.Ln
        ex = mybir.ActivationFunctionType.Exp
        out = {}
        for name, funcs in t.items():
            if name != "natural_log_exp_and_others" and (ln in funcs or ex in funcs):
                funcs = funcs - {ln, ex}
            out[name] = funcs
        return out

    hw_specs.get_activation_tables = patched
    bacc.get_activation_tables = patched
    hw_specs._act_tables_patched = True


def _build(nsh=NSH, blk=BLK):
    import concourse.bacc as bacc
    import concourse.tile as tile
    from concourse import mybir

    _patch_act_tables()

    f32 = mybir.dt.float32
    bf16 = mybir.dt.bfloat16
    f16 = mybir.dt.float16
    nblk = nsh // blk
    tp = blk // 128

    nc = bacc.Bacc("TRN2", target_bir_lowering=False, debug=False)
    xt = nc.dram_tensor("xt", [D, nsh], bf16, kind="ExternalInput").ap()
    cl = nc.dram_tensor("clusters", [K, D], f32, kind="ExternalInput").ap()
    qo = nc.dram_tensor("q", [nsh, K], bf16, kind="ExternalOutput").ap()

    with tile.TileContext(nc) as tc, ExitStack() as ctx:
        wp = ctx.enter_context(tc.tile_pool(name="w", bufs=1))
        io = ctx.enter_context(tc.tile_pool(name="io", bufs=4))
        sqp = ctx.enter_context(tc.tile_pool(name="sq", bufs=4))
        up = ctx.enter_context(tc.tile_pool(name="u", bufs=4))
        qp = ctx.enter_context(tc.tile_pool(name="qp", bufs=4))
        sp = ctx.enter_context(tc.tile_pool(name="sp", bufs=6))
        pp = ctx.enter_context(tc.tile_pool(name="ps", bufs=2, space="PSUM"))

        # --- one-time cluster prep (replicated on every core) ---
        cl_sb = wp.tile([K, D], f32, tag="cl")
        nc.sync.dma_start(cl_sb, cl)
        csq = wp.tile([K, D], f32, tag="csq")
        nc.vector.tensor_mul(csq, cl_sb, cl_sb)
        c2 = wp.tile([K, 1], f32, tag="c2")
        nc.vector.tensor_reduce(c2, csq, axis=mybir.AxisListType.X,
                                op=mybir.AluOpType.add)
        c2p1 = wp.tile([K, 1], f32, tag="c2p1")
        nc.vector.tensor_scalar_add(c2p1, c2, 1.0)
        cn2 = wp.tile([K, D], bf16, tag="cn2")
        nc.vector.tensor_scalar_mul(cn2, cl_sb, -2.0)
        ct0 = wp.tile([128, K], bf16, tag="ct0")
        ct1 = wp.tile([128, K], bf16, tag="ct1")
        nc.sync.dma_start_transpose(ct0, cn2[:, 0:128])
        nc.sync.dma_start_transpose(ct1, cn2[:, 128:256])
        ones = wp.tile([128, K], bf16, tag="ones")
        nc.vector.memset(ones, 1.0)

        for b in range(nblk):
            n0 = b * blk
            xt0 = io.tile([128, blk], bf16, tag="xt0")
            xt1 = io.tile([128, blk], bf16, tag="xt1")
            nc.sync.dma_start(xt0, xt[0:128, n0:n0 + blk])
            nc.sync.dma_start(xt1, xt[128:256, n0:n0 + blk])
            # squares for the x^2 row-norm term (gpsimd + scalar engines)
            xq0 = sqp.tile([128, blk], bf16, tag="xq0")
            xq1 = sqp.tile([128, blk], bf16, tag="xq1")
            nc.gpsimd.tensor_mul(xq0, xt0, xt0)
            nc.scalar.activation(xq1, xt1,
                                 func=mybir.ActivationFunctionType.Square)

            ps = pp.tile([K, blk], f32, tag="d2")
            for h in range(blk // 512):
                sl = slice(h * 512, h * 512 + 512)
                nc.tensor.matmul(ps[:, sl], ct0, xt0[:, sl], start=True, stop=False)
                nc.tensor.matmul(ps[:, sl], ct1, xt1[:, sl], start=False, stop=False)
                nc.tensor.matmul(ps[:, sl], ones, xq0[:, sl], start=False, stop=False)
                nc.tensor.matmul(ps[:, sl], ones, xq1[:, sl], start=False, stop=True)

            # u = ln(1 + d2) ; bias adds (1 + ||c_k||^2) per partition k
            u = up.tile([K, blk], f16, tag="u")
            nc.scalar.activation(u, ps, func=mybir.ActivationFunctionType.Ln,
                                 bias=c2p1, scale=1.0)

            # one xbar transpose to [n, k] layout
            ut = up.tile([128, tp, K], f16, tag="ut")
            nc.sync.dma_start_transpose(ut, u)

            # q = exp(-u)
            qb = qp.tile([128, tp, K], bf16, tag="qb")
            nc.scalar.activation(qb, ut,
                                 func=mybir.ActivationFunctionType.Exp, scale=-1.0)

            s = sp.tile([128, tp, 1], f32, tag="s")
            nc.vector.tensor_reduce(s, qb, axis=mybir.AxisListType.X,
                                    op=mybir.AluOpType.add)
            sinv = sp.tile([128, tp, 1], f32, tag="sinv")
            nc.vector.reciprocal(sinv, s)
            sinvb = sp.tile([128, tp, 1], bf16, tag="sinvb")
            nc.vector.tensor_copy(sinvb, sinv)

            qn = qp.tile([128, tp, K], bf16, tag="qn")
            nc.vector.tensor_tensor(qn, qb, sinvb.to_broadcast([128, tp, K]),
                                    op=mybir.AluOpType.mult)

            if XBAR_ORDER == "tp":
                q_blk = qo[n0:n0 + blk, :].rearrange("(t p) k -> p t k", p=128)
            else:
                q_blk = qo[n0:n0 + blk, :].rearrange("(p t) k -> p t k", t=tp)
            nc.sync.dma_start(q_blk, qn)

    nc.compile()
    return nc


_CACHE = {}


def _get_nc(nsh=NSH, blk=BLK):
    key = (nsh, blk)
    if key not in _CACHE:
        _CACHE[key] = _build(nsh, blk)
    return _CACHE[key]


def kernel(inputs, clusters):
    import ml_dtypes
    from concourse.bass_utils import run_bass_kernel_spmd

    x = np.asarray(inputs)
    c = np.ascontiguousarray(np.asarray(clusters, dtype=np.float32))
    assert x.shape == (N, D) and c.shape == (K, D)

    nc = _get_nc()
    xb = x.astype(ml_dtypes.bfloat16)
    in_maps = []
    for i in range(NCORES):
        xts = np.ascontiguousarray(xb[i * NSH:(i + 1) * NSH].T)  # [D, NSH] bf16
        in_maps.append({"xt": xts, "clusters": c})

    res = run_bass_kernel_spmd(nc, in_maps, core_ids=list(range(NCORES)))
    out = np.concatenate(
        [np.asarray(r["q"]).astype(np.float32) for r in res.results], axis=0
    )
    return out


# revision 5
# speedup vs baseline: 2.8470x; 1.3407x over previous
"""Trainium2 Bass kernel for nn_ClusteringLayer (vq_codebook, Student-t assignments).

Computes, for x in R^{N x D} and clusters c in R^{K x D}:
    d2[n,k] = ||x_n - c_k||^2
    q = 1 / (1 + d2)            (Student-t, alpha=1, power=(alpha+1)/2=1)
    out = q / q.sum(-1, keepdims=True)

Strategy (data-parallel over 8 NeuronCores, cluster table replicated):
  - host: shard x along N (65536 rows/core), transpose+cast each shard to
    bf16 X^T [D, Nsh] so the contraction dim lands on SBUF partitions.
  - device, per 1024-column block, with C^T stationary on the PE:
      psum[k, n] = (-2C)^T.T @ X^T  +  ones.T @ (X^T)^2     (8 matmuls of 512)
      u[k, n]    = Ln(psum + (1 + ||c||^2))   [ACT; per-partition bias; fp16]
      u^T        = one DMA-xbar transpose  -> [n, k] layout [128, 8, 64]
      q          = Exp(-u^T)                 [ACT; bf16]
      s          = row-sum over k (DVE reduce), out = q * (1/s) (DVE), store.
  - host: upcast bf16 -> f32, concat shards.
"""

import numpy as np
from contextlib import ExitStack

N, D, K = 524288, 256, 64
NCORES = 8
NSH = N // NCORES  # 65536 rows per core
BLK = 2048         # n-columns per block
# DMA-xbar transpose of [64, BLK] -> [128, BLK//128, 64]: which logical row
# ordering the xbar produces.  "tp" => row r = t*128 + p ; "pt" => r = p*tp + t
XBAR_ORDER = "tp"


def _patch_act_tables():
    """Make Ln and Exp resolve to the single set that contains both
    (natural_log_exp_and_others), so the kernel pays one ACT_TABLE_LOAD
    instead of alternating sets every block.  Only values are modified --
    set order (and hence act_func_set_id indices) is preserved."""
    import functools
    from concourse import hw_specs, bacc, mybir

    if getattr(hw_specs, "_act_tables_patched", False):
        return
    orig = hw_specs.get_activation_tables

    @functools.cache
    def patched(arch):
        t = dict(orig(arch))
        ln = mybir.ActivationFunctionType.Ln
        ex = mybir.ActivationFunctionType.Exp
        out = {}
        for name, funcs in t.items():
            if name != "natural_log_exp_and_others" and (ln in funcs or ex in funcs):
                funcs = funcs - {ln, ex}
            out[name] = funcs
        return out

    hw_specs.get_activation_tables = patched
    bacc.get_activation_tables = patched
    hw_specs._act_tables_patched = True


def _build(nsh=NSH, blk=BLK):
    import concourse.bacc as bacc
    import concourse.tile as tile
    from concourse import mybir

    _patch_act_tables()

    f32 = mybir.dt.float32
    bf16 = mybir.dt.bfloat16
    f16 = mybir.dt.float16
    nblk = nsh // blk
    tp = blk // 128

    nc = bacc.Bacc("TRN2", target_bir_lowering=False, debug=False)
    xt = nc.dram_tensor("xt", [D, nsh], bf16, kind="ExternalInput").ap()
    cl = nc.dram_tensor("clusters", [K, D], f32, kind="ExternalInput").ap()
    qo = nc.dram_tensor("q", [nsh, K], bf16, kind="ExternalOutput").ap()

    with tile.TileContext(nc) as tc, ExitStack() as ctx:
        wp = ctx.enter_context(tc.tile_pool(name="w", bufs=1))
        io = ctx.enter_context(tc.tile_pool(name="io", bufs=4))
        sqp = ctx.enter_context(tc.tile_pool(name="sq", bufs=4))
        up = ctx.enter_context(tc.tile_pool(name="u", bufs=4))
        qp = ctx.enter_context(tc.tile_pool(name="qp", bufs=4))
        sp = ctx.enter_context(tc.tile_pool(name="sp", bufs=6))
        pp = ctx.enter_context(tc.tile_pool(name="ps", bufs=2, space="PSUM"))

        # --- one-time cluster prep (replicated on every core) ---
        cl_sb = wp.tile([K, D], f32, tag="cl")
        nc.sync.dma_start(cl_sb, cl)
        csq = wp.tile([K, D], f32, tag="csq")
        nc.vector.tensor_mul(csq, cl_sb, cl_sb)
        c2 = wp.tile([K, 1], f32, tag="c2")
        nc.vector.tensor_reduce(c2, csq, axis=mybir.AxisListType.X,
                                op=mybir.AluOpType.add)
        c2p1 = wp.tile([K, 1], f32, tag="c2p1")
        nc.vector.tensor_scalar_add(c2p1, c2, 1.0)
        cn2 = wp.tile([K, D], bf16, tag="cn2")
        nc.vector.tensor_scalar_mul(cn2, cl_sb, -2.0)
        ct0 = wp.tile([128, K], bf16, tag="ct0")
        ct1 = wp.tile([128, K], bf16, tag="ct1")
        nc.sync.dma_start_transpose(ct0, cn2[:, 0:128])
        nc.sync.dma_start_transpose(ct1, cn2[:, 128:256])
        ones = wp.tile([128, K], bf16, tag="ones")
        nc.vector.memset(ones, 1.0)

        for b in range(nblk):
            n0 = b * blk
            xt0 = io.tile([128, blk], bf16, tag="xt0")
            xt1 = io.tile([128, blk], bf16, tag="xt1")
            nc.sync.dma_start(xt0, xt[0:128, n0:n0 + blk])
            nc.sync.dma_start(xt1, xt[128:256, n0:n0 + blk])
            # squares for the x^2 row-norm term (gpsimd + scalar engines)
            xq0 = sqp.tile([128, blk], bf16, tag="xq0")
            xq1 = sqp.tile([128, blk], bf16, tag="xq1")
            nc.gpsimd.tensor_mul(xq0, xt0, xt0)
            nc.scalar.activation(xq1, xt1,
                                 func=mybir.ActivationFunctionType.Square)

            ps = pp.tile([K, blk], f32, tag="d2")
            for h in range(blk // 512):
                sl = slice(h * 512, h * 512 + 512)
                nc.tensor.matmul(ps[:, sl], ct0, xt0[:, sl], start=True, stop=False)
                nc.tensor.matmul(ps[:, sl], ct1, xt1[:, sl], start=False, stop=False)
                nc.tensor.matmul(ps[:, sl], ones, xq0[:, sl], start=False, stop=False)
                nc.tensor.matmul(ps[:, sl], ones, xq1[:, sl], start=False, stop=True)

            # u = ln(1 + d2) ; bias adds (1 + ||c_k||^2) per partition k
            u = up.tile([K, blk], f16, tag="u")
            nc.scalar.activation(u, ps, func=mybir.ActivationFunctionType.Ln,
                                 bias=c2p1, scale=1.0)

            # one xbar transpose to [n, k] layout.  Issued from the Scalar
            # engine's HWDGE queue so its wait-for-Ln doesn't head-of-line
            # block the loads/stores queued on the Sync engine.
            ut = up.tile([128, tp, K], f16, tag="ut")
            nc.scalar.dma_start_transpose(ut, u)

            # q = exp(-u)
            qb = qp.tile([128, tp, K], bf16, tag="qb")
            nc.scalar.activation(qb, ut,
                                 func=mybir.ActivationFunctionType.Exp, scale=-1.0)

            s = sp.tile([128, tp, 1], f32, tag="s")
            nc.vector.tensor_reduce(s, qb, axis=mybir.AxisListType.X,
                                    op=mybir.AluOpType.add)
            sinv = sp.tile([128, tp, 1], f32, tag="sinv")
            nc.vector.reciprocal(sinv, s)
            sinvb = sp.tile([128, tp, 1], bf16, tag="sinvb")
            nc.vector.tensor_copy(sinvb, sinv)

            qn = qp.tile([128, tp, K], bf16, tag="qn")
            nc.vector.tensor_tensor(qn, qb, sinvb.to_broadcast([128, tp, K]),
                                    op=mybir.AluOpType.mult)

            if XBAR_ORDER == "tp":
                q_blk = qo[n0:n0 + blk, :].rearrange("(t p) k -> p t k", p=128)
            else:
                q_blk = qo[n0:n0 + blk, :].rearrange("(p t) k -> p t k", t=tp)
            nc.sync.dma_start(q_blk, qn)

    nc.compile()
    return nc


_CACHE = {}


def _get_nc(nsh=NSH, blk=BLK):
    key = (nsh, blk)
    if key not in _CACHE:
        _CACHE[key] = _build(nsh, blk)
    return _CACHE[key]


def kernel(inputs, clusters):
    import ml_dtypes
    from concourse.bass_utils import run_bass_kernel_spmd

    x = np.asarray(inputs)
    c = np.ascontiguousarray(np.asarray(clusters, dtype=np.float32))
    assert x.shape == (N, D) and c.shape == (K, D)

    nc = _get_nc()
    xb = x.astype(ml_dtypes.bfloat16)
    in_maps = []
    for i in range(NCORES):
        xts = np.ascontiguousarray(xb[i * NSH:(i + 1) * NSH].T)  # [D, NSH] bf16
        in_maps.append({"xt": xts, "clusters": c})

    res = run_bass_kernel_spmd(nc, in_maps, core_ids=list(range(NCORES)))
    out = np.concatenate(
        [np.asarray(r["q"]).astype(np.float32) for r in res.results], axis=0
    )
    return out
